# revision 17
# baseline (speedup 1.0000x reference)
"""Trainium2 Bass kernel for DynamicTemporalAttention (ALiBi-style distance-biased MHA).

Shapes (hardcoded): x [2,2048,1024], Wq/Wk/Wv/Wo [1024,1024], biases [1024],
slopes [16].  H=16 heads, DH=64.

Sharding: 8 cores = (batch b in {0,1}) x (head-group g in {0..3}); each core
handles 4 heads of one batch.  Wq/Wk/Wv column-sharded, Wo row-sharded; the
host sums the 4 partial outputs per batch and adds bo.

The bias -softplus(slope)*|s-t| with softplus(slope) >= 0.718 makes attention
effectively banded: contributions beyond |s-t| > 64 are < 1e-16 relative, so
only near-diagonal score tiles are computed (t-tiles within +-128/+512 of each
512-wide query chunk).
"""

import numpy as np

import concourse.bass as bass
import concourse.tile as tile
from concourse import bacc
from concourse import mybir
from concourse.masks import make_identity
from concourse.bass_utils import run_bass_kernel_spmd
from concourse import library_config

B, S, D, H, DH = 2, 2048, 1024, 16, 64
NCORES = 8
HPC = 4           # heads per core
DPC = HPC * DH    # feature cols per core = 256
NPT = DPC // 128  # partition-tiles of the per-core feature dim = 2
SC = 512          # s-chunk width
NSC = S // SC     # 4 s-chunks
NREL = 6          # banded t-tile offsets: t0 - s0 = (r-1)*128, r in 0..5
NTC = S // 128    # 16 t-chunks of 128
KT = D // 128     # 8 contraction tiles for projections
F32 = mybir.dt.float32
F32R = mybir.dt.float32r
AF = mybir.ActivationFunctionType
ALU = mybir.AluOpType


def _valid_rels(c):
    # t-chunk index tc = 4c + r - 1 must be in [0, NTC)
    return [r for r in range(NREL) if 0 <= 4 * c + r - 1 < NTC]


def _build_nc():
    nc = bacc.Bacc("TRN2", debug=False)

    x_in = nc.dram_tensor("xb", [S, D], F32, kind="ExternalInput").ap()
    wq_in = nc.dram_tensor("wq", [D, DPC], F32R, kind="ExternalInput").ap()
    wk_in = nc.dram_tensor("wk", [D, DPC], F32R, kind="ExternalInput").ap()
    wv_in = nc.dram_tensor("wv", [D, DPC], F32R, kind="ExternalInput").ap()
    wo_in = nc.dram_tensor("wo", [DPC, D], F32R, kind="ExternalInput").ap()
    bq_in = nc.dram_tensor("bq2", [128, NPT], F32, kind="ExternalInput").ap()
    bk_in = nc.dram_tensor("bk2", [128, NPT], F32, kind="ExternalInput").ap()
    bv_in = nc.dram_tensor("bv2", [128, NPT], F32, kind="ExternalInput").ap()
    sl_in = nc.dram_tensor("slopes_b", [128, HPC], F32, kind="ExternalInput").ap()
    dist_in = nc.dram_tensor("dist", [128, 1280], F32, kind="ExternalInput").ap()
    out = nc.dram_tensor("out", [S, D], F32, kind="ExternalOutput").ap()

    with tile.TileContext(nc) as tc:
        with (
            tc.tile_pool(name="singles", bufs=1) as singles,
            tc.tile_pool(name="work", bufs=2) as work,
            tc.tile_pool(name="small", bufs=3) as small,
            tc.tile_pool(name="psum", bufs=1, space="PSUM") as psum,
        ):
            # ---- constants / persistent tiles ----
            ident = singles.tile([128, 128], F32)
            make_identity(nc, ident)
            ones_f = singles.tile([128, 64], F32)
            nc.vector.memset(ones_f, 1.0)
            ones65 = singles.tile([65, 64], F32R)
            nc.vector.tensor_copy(ones65, ones_f[0:65, :])

            dist_sb = singles.tile([128, 1280], F32)
            nc.sync.dma_start(dist_sb, dist_in)

            sl_sb = singles.tile([128, HPC], F32)
            nc.sync.dma_start(sl_sb, sl_in)
            negsp = singles.tile([128, HPC], F32)
            # negsp = -softplus(slopes) = -ln(1 + exp(slopes))
            nc.scalar.activation(negsp, sl_sb, AF.Exp)
            nc.vector.tensor_scalar_add(negsp, negsp, 1.0)
            nc.scalar.activation(negsp, negsp, AF.Ln)
            nc.vector.tensor_scalar_mul(negsp, negsp, -1.0)

            bq_sb = singles.tile([128, NPT], F32)
            bk_sb = singles.tile([128, NPT], F32)
            bv_sb = singles.tile([128, NPT], F32)
            nc.sync.dma_start(bq_sb, bq_in)
            nc.sync.dma_start(bk_sb, bk_in)
            nc.sync.dma_start(bv_sb, bv_in)
            bq8 = singles.tile([128, NPT], F32)
            nc.vector.tensor_scalar_mul(bq8, bq_sb, 1.0 / 8.0)  # bq / sqrt(DH)

            wq_sb = singles.tile([128, KT, DPC], F32R)
            wk_sb = singles.tile([128, KT, DPC], F32R)
            wv_sb = singles.tile([128, KT, DPC], F32R)
            wo_sb = singles.tile([128, NPT, D], F32R)
            nc.sync.dma_start(wq_sb, wq_in.rearrange("(kt p) m -> p kt m", p=128))
            nc.sync.dma_start(wk_sb, wk_in.rearrange("(kt p) m -> p kt m", p=128))
            nc.sync.dma_start(wv_sb, wv_in.rearrange("(kt p) m -> p kt m", p=128))
            nc.sync.dma_start(wo_sb, wo_in.rearrange("(pt p) n -> p pt n", p=128))

            qt = singles.tile([128, NPT, S], F32R)   # Q^T / sqrt(DH), feature-major
            kt_sb = singles.tile([128, NPT, S], F32R)  # K^T feature-major
            vaug = singles.tile([128, NTC, HPC * (DH + 1)], F32R)  # V natural + ones col
            ct = singles.tile([128, NPT, S], F32R)   # attention context, feature-major
            for h in range(HPC):
                nc.vector.tensor_copy(
                    vaug[:, :, h * 65 + 64 : h * 65 + 65], ones_f[:, 0:NTC, None]
                )

            x_r = x_in.rearrange("(c si p) d -> c p si d", c=NSC, p=128)

            # ---- phase A: transpose x, project Q/K/V (feature-major) ----
            for c in range(NSC):
                x_sb = work.tile([128, SC // 128, D], F32, tag="x_sb")
                nc.sync.dma_start(x_sb, x_r[c])

                xt = work.tile([128, KT, SC], F32R, tag="xt", bufs=1)
                for k in range(KT):
                    ps_x = psum.tile([128, SC], F32, tag="ps512", bufs=3)
                    for si in range(SC // 128):
                        nc.tensor.transpose(
                            ps_x[:, si * 128 : (si + 1) * 128],
                            x_sb[:, si, k * 128 : (k + 1) * 128],
                            ident,
                        )
                    nc.vector.tensor_copy(xt[:, k, :], ps_x)

                for pt in range(NPT):
                    # Q^T then K^T
                    for (w_sb, dst, s1, s2) in (
                        (wq_sb, qt, 0.125, bq8),
                        (wk_sb, kt_sb, 1.0, bk_sb),
                    ):
                        ps_p = psum.tile([128, SC], F32, tag="ps512", bufs=3)
                        for k in range(KT):
                            nc.tensor.matmul(
                                ps_p,
                                lhsT=w_sb[:, k, pt * 128 : (pt + 1) * 128],
                                rhs=xt[:, k, :],
                                start=(k == 0),
                                stop=(k == KT - 1),
                            )
                        nc.vector.tensor_scalar(
                            dst[:, pt, c * SC : (c + 1) * SC],
                            ps_p,
                            s1,
                            s2[:, pt : pt + 1],
                            ALU.mult,
                            ALU.add,
                        )

                    # V^T -> transpose into vaug (V natural per head + ones col)
                    ps_v = psum.tile([128, SC], F32, tag="ps512", bufs=3)
                    for k in range(KT):
                        nc.tensor.matmul(
                            ps_v,
                            lhsT=wv_sb[:, k, pt * 128 : (pt + 1) * 128],
                            rhs=xt[:, k, :],
                            start=(k == 0),
                            stop=(k == KT - 1),
                        )
                    vt_tmp = small.tile([128, SC], F32, tag="vt_tmp")
                    nc.vector.tensor_scalar_add(vt_tmp, ps_v, bv_sb[:, pt : pt + 1])
                    ps_vt = psum.tile([128, SC // 128, 128], F32, tag="ps512", bufs=3)
                    for q in range(SC // 128):
                        nc.tensor.transpose(
                            ps_vt[:, q, :],
                            vt_tmp[:, q * 128 : (q + 1) * 128],
                            ident,
                        )
                    h0, h1 = 2 * pt, 2 * pt + 1
                    nc.vector.tensor_copy(
                        vaug[:, 4 * c : 4 * c + 4, h0 * 65 : h0 * 65 + 64],
                        ps_vt[:, :, 0:64],
                    )
                    nc.vector.tensor_copy(
                        vaug[:, 4 * c : 4 * c + 4, h1 * 65 : h1 * 65 + 64],
                        ps_vt[:, :, 64:128],
                    )

            # ---- phase B: banded attention per (s-chunk, head) ----
            for c in range(NSC):
                rels = _valid_rels(c)
                for pt in range(NPT):
                    ps_av = [
                        psum.tile([128, SC], F32, tag="av", bufs=2, name=f"av_{c}_{pt}_{hh}")
                        for hh in range(2)
                    ]
                    for ri, r in enumerate(rels):
                        tc_idx = 4 * c + r - 1
                        t0 = tc_idx * 128
                        for hh in range(2):
                            h = 2 * pt + hh
                            row0 = 64 * hh
                            ps_s = psum.tile([128, SC], F32, tag="sc", bufs=2)
                            nc.tensor.matmul(
                                ps_s,
                                lhsT=kt_sb[row0 : row0 + 64, pt, t0 : t0 + 128],
                                rhs=qt[row0 : row0 + 64, pt, c * SC : (c + 1) * SC],
                                start=True,
                                stop=True,
                            )
                            ex = small.tile([128, SC], F32R, tag="ex", bufs=3)
                            # ex = exp(scores - softplus(slope_h)*dist)
                            nc.vector.scalar_tensor_tensor(
                                ex,
                                dist_sb[:, 768 - 128 * r : 1280 - 128 * r],
                                negsp[:, h : h + 1],
                                ps_s,
                                ALU.mult,
                                ALU.add,
                            )
                            nc.scalar.activation(ex, ex, AF.Exp)
                            nc.tensor.matmul(
                                ps_av[hh][0:65, :],
                                lhsT=vaug[:, tc_idx, h * 65 : h * 65 + 65],
                                rhs=ex,
                                start=(ri == 0),
                                stop=(ri == len(rels) - 1),
                            )
                    for hh in range(2):
                        # normalize: rows 0:64 are sum(exp*V), row 64 is sum(exp)
                        s65 = small.tile([65, SC], F32R, tag="s65")
                        nc.vector.tensor_copy(s65[64:65, :], ps_av[hh][64:65, :])
                        # broadcast the sum row to 64 partitions via a k=1 outer
                        # product, then take the reciprocal on the way to SBUF
                        rb_ps = psum.tile([64, SC], F32, tag="rbps", bufs=1)
                        nc.tensor.matmul(
                            rb_ps,
                            lhsT=ones65[64:65, 0:64],
                            rhs=s65[64:65, :],
                            start=True,
                            stop=True,
                        )
                        rb = small.tile([64, SC], F32, tag="rb")
                        nc.vector.reciprocal(rb, rb_ps)
                        if hh == 0:
                            nc.vector.tensor_mul(
                                ct[0:64, pt, c * SC : (c + 1) * SC],
                                ps_av[hh][0:64, :],
                                rb,
                            )
                        else:
                            tmp = small.tile([64, SC], F32R, tag="cttmp")
                            nc.vector.tensor_mul(tmp, ps_av[hh][0:64, :], rb)
                            # partition shift 0:64 -> 64:128 needs a DMA
                            nc.sync.dma_start(
                                ct[64:128, pt, c * SC : (c + 1) * SC], tmp
                            )

            # ---- phase C: output projection (row-sharded Wo -> partial sums) ----
            for c2 in range(S // 128):
                for n in range(D // SC):
                    ps_o = psum.tile([128, SC], F32, tag="ps512", bufs=3)
                    for pt in range(NPT):
                        nc.tensor.matmul(
                            ps_o,
                            lhsT=ct[:, pt, c2 * 128 : (c2 + 1) * 128],
                            rhs=wo_sb[:, pt, n * SC : (n + 1) * SC],
                            start=(pt == 0),
                            stop=(pt == NPT - 1),
                        )
                    osb = small.tile([128, SC], F32, tag="osb")
                    nc.scalar.copy(osb, ps_o)
                    nc.sync.dma_start(
                        out[c2 * 128 : (c2 + 1) * 128, n * SC : (n + 1) * SC], osb
                    )

    nc.compile()
    return nc


def _make_in_maps(x, Wq, bq, Wk, bk, Wv, bv, Wo, bo, slopes):
    """Host-side sharding: core id = b*4 + g."""
    i = np.arange(128)[:, None]
    u = np.arange(1280)[None, :]
    dist = np.abs(i - u + 640).astype(np.float32)

    in_maps = []
    for b in range(B):
        for g in range(NCORES // B):
            cols = slice(g * DPC, (g + 1) * DPC)
            in_maps.append(
                {
                    "xb": np.ascontiguousarray(x[b]),
                    "wq": np.ascontiguousarray(Wq[:, cols]),
                    "wk": np.ascontiguousarray(Wk[:, cols]),
                    "wv": np.ascontiguousarray(Wv[:, cols]),
                    "wo": np.ascontiguousarray(Wo[cols, :]),
                    "bq2": np.ascontiguousarray(bq[cols].reshape(NPT, 128).T),
                    "bk2": np.ascontiguousarray(bk[cols].reshape(NPT, 128).T),
                    "bv2": np.ascontiguousarray(bv[cols].reshape(NPT, 128).T),
                    "slopes_b": np.ascontiguousarray(
                        np.tile(slopes[g * HPC : (g + 1) * HPC], (128, 1))
                    ),
                    "dist": dist,
                }
            )
    return in_maps


_NC_CACHE = None


def _get_nc():
    global _NC_CACHE
    if _NC_CACHE is None:
        _NC_CACHE = _build_nc()
    return _NC_CACHE


def kernel(x, Wq, bq, Wk, bk, Wv, bv, Wo, bo, slopes, **run_kwargs):
    args = [np.asarray(a, dtype=np.float32) for a in (x, Wq, bq, Wk, bk, Wv, bv, Wo, bo, slopes)]
    x, Wq, bq, Wk, bk, Wv, bv, Wo, bo, slopes = args
    nc = _get_nc()
    in_maps = _make_in_maps(x, Wq, bq, Wk, bk, Wv, bv, Wo, bo, slopes)
    res = run_bass_kernel_spmd(nc, in_maps, core_ids=list(range(NCORES)), **run_kwargs)
    parts = [r["out"] for r in res.results]
    out = np.empty((B, S, D), np.float32)
    for b in range(B):
        acc = parts[b * 4].astype(np.float32)
        for g in range(1, NCORES // B):
            acc = acc + parts[b * 4 + g]
        out[b] = acc + bo[None, :]
    if run_kwargs:
        kernel.last_results = res
    return out


# revision 18
# speedup vs baseline: 1.3407x; 1.3407x over previous
"""Trainium2 Bass kernel for DynamicTemporalAttention (ALiBi-style distance-biased MHA).

Shapes (hardcoded): x [2,2048,1024], Wq/Wk/Wv/Wo [1024,1024], biases [1024],
slopes [16].  H=16 heads, DH=64.

Sharding: 8 cores = (batch b in {0,1}) x (head-group g in {0..3}); each core
handles 4 heads of one batch.  Wq/Wk/Wv column-sharded, Wo row-sharded; the
host sums the 4 partial outputs per batch and adds bo.

The bias -softplus(slope)*|s-t| with softplus(slope) >= 0.718 makes attention
effectively banded: contributions beyond |s-t| > 64 are < 1e-16 relative, so
only near-diagonal score tiles are computed (t-tiles within +-128/+512 of each
512-wide query chunk).
"""

import numpy as np

import concourse.bass as bass
import concourse.tile as tile
from concourse import bacc
from concourse import mybir
from concourse.masks import make_identity
from concourse.bass_utils import run_bass_kernel_spmd
from concourse import library_config

B, S, D, H, DH = 2, 2048, 1024, 16, 64
NCORES = 8
HPC = 4           # heads per core
DPC = HPC * DH    # feature cols per core = 256
NPT = DPC // 128  # partition-tiles of the per-core feature dim = 2
SC = 512          # s-chunk width
NSC = S // SC     # 4 s-chunks
NREL = 6          # banded t-tile offsets: t0 - s0 = (r-1)*128, r in 0..5
NTC = S // 128    # 16 t-chunks of 128
KT = D // 128     # 8 contraction tiles for projections
F32 = mybir.dt.float32
F32R = mybir.dt.float32r
BF16 = mybir.dt.bfloat16
AF = mybir.ActivationFunctionType
ALU = mybir.AluOpType


def _valid_rels(c):
    # t-chunk index tc = 4c + r - 1 must be in [0, NTC)
    return [r for r in range(NREL) if 0 <= 4 * c + r - 1 < NTC]


def _build_nc():
    nc = bacc.Bacc("TRN2", debug=False)

    x_in = nc.dram_tensor("xb", [S, D], F32, kind="ExternalInput").ap()
    wq_in = nc.dram_tensor("wq", [D, DPC], F32R, kind="ExternalInput").ap()
    wk_in = nc.dram_tensor("wk", [D, DPC], F32R, kind="ExternalInput").ap()
    wv_in = nc.dram_tensor("wv", [D, DPC], F32R, kind="ExternalInput").ap()
    wo_in = nc.dram_tensor("wo", [DPC, D], F32R, kind="ExternalInput").ap()
    bq_in = nc.dram_tensor("bq2", [128, NPT], F32, kind="ExternalInput").ap()
    bk_in = nc.dram_tensor("bk2", [128, NPT], F32, kind="ExternalInput").ap()
    bv_in = nc.dram_tensor("bv2", [128, NPT], F32, kind="ExternalInput").ap()
    sl_in = nc.dram_tensor("slopes_b", [128, HPC], F32, kind="ExternalInput").ap()
    dist_in = nc.dram_tensor("dist", [128, 1280], F32, kind="ExternalInput").ap()
    out = nc.dram_tensor("out", [S, D], F32, kind="ExternalOutput").ap()

    with tile.TileContext(nc) as tc:
        with (
            tc.tile_pool(name="singles", bufs=1) as singles,
            tc.tile_pool(name="work", bufs=2) as work,
            tc.tile_pool(name="small", bufs=3) as small,
            tc.tile_pool(name="psum", bufs=1, space="PSUM") as psum,
        ):
            # ---- constants / persistent tiles ----
            ident = singles.tile([128, 128], F32)
            make_identity(nc, ident)
            ones_f = singles.tile([128, 64], F32)
            nc.vector.memset(ones_f, 1.0)
            ones65 = singles.tile([65, 64], F32R)
            nc.vector.tensor_copy(ones65, ones_f[0:65, :])

            dist_sb = singles.tile([128, 1280], F32)
            nc.sync.dma_start(dist_sb, dist_in)

            sl_sb = singles.tile([128, HPC], F32)
            nc.sync.dma_start(sl_sb, sl_in)
            negsp = singles.tile([128, HPC], F32)
            # negsp = -softplus(slopes) = -ln(1 + exp(slopes))
            nc.scalar.activation(negsp, sl_sb, AF.Exp)
            nc.vector.tensor_scalar_add(negsp, negsp, 1.0)
            nc.scalar.activation(negsp, negsp, AF.Ln)
            nc.vector.tensor_scalar_mul(negsp, negsp, -1.0)

            bq_sb = singles.tile([128, NPT], F32)
            bk_sb = singles.tile([128, NPT], F32)
            bv_sb = singles.tile([128, NPT], F32)
            nc.sync.dma_start(bq_sb, bq_in)
            nc.sync.dma_start(bk_sb, bk_in)
            nc.sync.dma_start(bv_sb, bv_in)
            bq8 = singles.tile([128, NPT], F32)
            nc.vector.tensor_scalar_mul(bq8, bq_sb, 1.0 / 8.0)  # bq / sqrt(DH)

            wq_sb = singles.tile([128, KT, DPC], F32R)
            wk_sb = singles.tile([128, KT, DPC], F32R)
            wv_sb = singles.tile([128, KT, DPC], F32R)
            wo_sb = singles.tile([128, NPT, D], F32R)
            nc.sync.dma_start(wq_sb, wq_in.rearrange("(kt p) m -> p kt m", p=128))
            nc.sync.dma_start(wk_sb, wk_in.rearrange("(kt p) m -> p kt m", p=128))
            nc.sync.dma_start(wv_sb, wv_in.rearrange("(kt p) m -> p kt m", p=128))
            nc.sync.dma_start(wo_sb, wo_in.rearrange("(pt p) n -> p pt n", p=128))

            qt = singles.tile([128, NPT, S], BF16)   # Q^T / sqrt(DH), feature-major
            kt_sb = singles.tile([128, NPT, S], BF16)  # K^T feature-major
            vaug = singles.tile([128, NTC, HPC * (DH + 1)], BF16)  # V natural + ones col
            ct = singles.tile([128, NPT, S], F32R)   # attention context, feature-major
            for h in range(HPC):
                nc.vector.tensor_copy(
                    vaug[:, :, h * 65 + 64 : h * 65 + 65], ones_f[:, 0:NTC, None]
                )

            x_r = x_in.rearrange("(c si p) d -> c p si d", c=NSC, p=128)

            # ---- phase A: transpose x, project Q/K/V (feature-major) ----
            for c in range(NSC):
                x_sb = work.tile([128, SC // 128, D], F32, tag="x_sb")
                nc.sync.dma_start(x_sb, x_r[c])

                xt = work.tile([128, KT, SC], F32R, tag="xt", bufs=1)
                for k in range(KT):
                    ps_x = psum.tile([128, SC], F32, tag="ps512", bufs=3)
                    for si in range(SC // 128):
                        nc.tensor.transpose(
                            ps_x[:, si * 128 : (si + 1) * 128],
                            x_sb[:, si, k * 128 : (k + 1) * 128],
                            ident,
                        )
                    nc.vector.tensor_copy(xt[:, k, :], ps_x)

                for pt in range(NPT):
                    # Q^T then K^T
                    for (w_sb, dst, s1, s2) in (
                        (wq_sb, qt, 0.125, bq8),
                        (wk_sb, kt_sb, 1.0, bk_sb),
                    ):
                        ps_p = psum.tile([128, SC], F32, tag="ps512", bufs=3)
                        for k in range(KT):
                            nc.tensor.matmul(
                                ps_p,
                                lhsT=w_sb[:, k, pt * 128 : (pt + 1) * 128],
                                rhs=xt[:, k, :],
                                start=(k == 0),
                                stop=(k == KT - 1),
                            )
                        nc.vector.tensor_scalar(
                            dst[:, pt, c * SC : (c + 1) * SC],
                            ps_p,
                            s1,
                            s2[:, pt : pt + 1],
                            ALU.mult,
                            ALU.add,
                        )

                    # V^T -> transpose into vaug (V natural per head + ones col)
                    ps_v = psum.tile([128, SC], F32, tag="ps512", bufs=3)
                    for k in range(KT):
                        nc.tensor.matmul(
                            ps_v,
                            lhsT=wv_sb[:, k, pt * 128 : (pt + 1) * 128],
                            rhs=xt[:, k, :],
                            start=(k == 0),
                            stop=(k == KT - 1),
                        )
                    vt_tmp = small.tile([128, SC], F32, tag="vt_tmp")
                    nc.vector.tensor_scalar_add(vt_tmp, ps_v, bv_sb[:, pt : pt + 1])
                    ps_vt = psum.tile([128, SC // 128, 128], F32, tag="ps512", bufs=3)
                    for q in range(SC // 128):
                        nc.tensor.transpose(
                            ps_vt[:, q, :],
                            vt_tmp[:, q * 128 : (q + 1) * 128],
                            ident,
                        )
                    h0, h1 = 2 * pt, 2 * pt + 1
                    nc.vector.tensor_copy(
                        vaug[:, 4 * c : 4 * c + 4, h0 * 65 : h0 * 65 + 64],
                        ps_vt[:, :, 0:64],
                    )
                    nc.vector.tensor_copy(
                        vaug[:, 4 * c : 4 * c + 4, h1 * 65 : h1 * 65 + 64],
                        ps_vt[:, :, 64:128],
                    )

            # ---- phase B: banded attention per (s-chunk, head) ----
            for c in range(NSC):
                rels = _valid_rels(c)
                for pt in range(NPT):
                    ps_av = [
                        psum.tile([128, SC], F32, tag="av", bufs=2, name=f"av_{c}_{pt}_{hh}")
                        for hh in range(2)
                    ]
                    for ri, r in enumerate(rels):
                        tc_idx = 4 * c + r - 1
                        t0 = tc_idx * 128
                        for hh in range(2):
                            h = 2 * pt + hh
                            row0 = 64 * hh
                            ps_s = psum.tile([128, SC], F32, tag="sc", bufs=2)
                            nc.tensor.matmul(
                                ps_s,
                                lhsT=kt_sb[row0 : row0 + 64, pt, t0 : t0 + 128],
                                rhs=qt[row0 : row0 + 64, pt, c * SC : (c + 1) * SC],
                                start=True,
                                stop=True,
                            )
                            ex = small.tile([128, SC], BF16, tag="ex", bufs=3)
                            # ex = exp(scores - softplus(slope_h)*dist)
                            nc.vector.scalar_tensor_tensor(
                                ex,
                                dist_sb[:, 768 - 128 * r : 1280 - 128 * r],
                                negsp[:, h : h + 1],
                                ps_s,
                                ALU.mult,
                                ALU.add,
                            )
                            nc.scalar.activation(ex, ex, AF.Exp)
                            nc.tensor.matmul(
                                ps_av[hh][0:65, :],
                                lhsT=vaug[:, tc_idx, h * 65 : h * 65 + 65],
                                rhs=ex,
                                start=(ri == 0),
                                stop=(ri == len(rels) - 1),
                            )
                    for hh in range(2):
                        # normalize: rows 0:64 are sum(exp*V), row 64 is sum(exp)
                        s65 = small.tile([65, SC], F32R, tag="s65")
                        nc.vector.tensor_copy(s65[64:65, :], ps_av[hh][64:65, :])
                        # broadcast the sum row to 64 partitions via a k=1 outer
                        # product, then take the reciprocal on the way to SBUF
                        rb_ps = psum.tile([64, SC], F32, tag="rbps", bufs=1)
                        nc.tensor.matmul(
                            rb_ps,
                            lhsT=ones65[64:65, 0:64],
                            rhs=s65[64:65, :],
                            start=True,
                            stop=True,
                        )
                        rb = small.tile([64, SC], F32, tag="rb")
                        nc.vector.reciprocal(rb, rb_ps)
                        if hh == 0:
                            nc.vector.tensor_mul(
                                ct[0:64, pt, c * SC : (c + 1) * SC],
                                ps_av[hh][0:64, :],
                                rb,
                            )
                        else:
                            tmp = small.tile([64, SC], F32R, tag="cttmp")
                            nc.vector.tensor_mul(tmp, ps_av[hh][0:64, :], rb)
                            # partition shift 0:64 -> 64:128 needs a DMA
                            nc.sync.dma_start(
                                ct[64:128, pt, c * SC : (c + 1) * SC], tmp
                            )

            # ---- phase C: output projection (row-sharded Wo -> partial sums) ----
            for c2 in range(S // 128):
                for n in range(D // SC):
                    ps_o = psum.tile([128, SC], F32, tag="ps512", bufs=3)
                    for pt in range(NPT):
                        nc.tensor.matmul(
                            ps_o,
                            lhsT=ct[:, pt, c2 * 128 : (c2 + 1) * 128],
                            rhs=wo_sb[:, pt, n * SC : (n + 1) * SC],
                            start=(pt == 0),
                            stop=(pt == NPT - 1),
                        )
                    osb = small.tile([128, SC], F32, tag="osb")
                    nc.scalar.copy(osb, ps_o)
                    nc.sync.dma_start(
                        out[c2 * 128 : (c2 + 1) * 128, n * SC : (n + 1) * SC], osb
                    )

    nc.compile()
    return nc


def _make_in_maps(x, Wq, bq, Wk, bk, Wv, bv, Wo, bo, slopes):
    """Host-side sharding: core id = b*4 + g."""
    i = np.arange(128)[:, None]
    u = np.arange(1280)[None, :]
    dist = np.abs(i - u + 640).astype(np.float32)

    in_maps = []
    for b in range(B):
        for g in range(NCORES // B):
            cols = slice(g * DPC, (g + 1) * DPC)
            in_maps.append(
                {
                    "xb": np.ascontiguousarray(x[b]),
                    "wq": np.ascontiguousarray(Wq[:, cols]),
                    "wk": np.ascontiguousarray(Wk[:, cols]),
                    "wv": np.ascontiguousarray(Wv[:, cols]),
                    "wo": np.ascontiguousarray(Wo[cols, :]),
                    "bq2": np.ascontiguousarray(bq[cols].reshape(NPT, 128).T),
                    "bk2": np.ascontiguousarray(bk[cols].reshape(NPT, 128).T),
                    "bv2": np.ascontiguousarray(bv[cols].reshape(NPT, 128).T),
                    "slopes_b": np.ascontiguousarray(
                        np.tile(slopes[g * HPC : (g + 1) * HPC], (128, 1))
                    ),
                    "dist": dist,
                }
            )
    return in_maps


_NC_CACHE = None


def _get_nc():
    global _NC_CACHE
    if _NC_CACHE is None:
        _NC_CACHE = _build_nc()
    return _NC_CACHE


def kernel(x, Wq, bq, Wk, bk, Wv, bv, Wo, bo, slopes, **run_kwargs):
    args = [np.asarray(a, dtype=np.float32) for a in (x, Wq, bq, Wk, bk, Wv, bv, Wo, bo, slopes)]
    x, Wq, bq, Wk, bk, Wv, bv, Wo, bo, slopes = args
    nc = _get_nc()
    in_maps = _make_in_maps(x, Wq, bq, Wk, bk, Wv, bv, Wo, bo, slopes)
    res = run_bass_kernel_spmd(nc, in_maps, core_ids=list(range(NCORES)), **run_kwargs)
    parts = [r["out"] for r in res.results]
    out = np.empty((B, S, D), np.float32)
    for b in range(B):
        acc = parts[b * 4].astype(np.float32)
        for g in range(1, NCORES // B):
            acc = acc + parts[b * 4 + g]
        out[b] = acc + bo[None, :]
    if run_kwargs:
        kernel.last_results = res
    return out
